# revision 21
# baseline (speedup 1.0000x reference)
"""Trainium2 Bass kernel for batched multi-head attention with deterministic dropout.

Reference computation (B=2, H=16, S=2048, D=128, fp32):
    qk   = einsum("bhqd,bhkd->bhqk", q, k)          # scores
    attn = softmax(qk, axis=-1)
    keep = bernoulli(jax.random.key(42), 0.9, attn.shape)
    attn = where(keep, attn / 0.9, 0)
    out  = einsum("bhqk,bhkd->bhqd", attn, v)

Sharding: the 32 (b,h) pairs are split across 8 NeuronCores, 4 pairs each.
Device-side pipeline per (pair, q-block) — everything in the S^T orientation so
no transposes of the 2048x2048 attention matrix are ever needed:
    S^T[k,q]   = K·Q^T                 (PE, fp16 in, fp32 PSUM)
    E^T        = exp(S^T)              (ACT, bf16 out; scores max ~62 so no
                                        max-subtraction is needed in fp32/bf16 range)
    rowsum_bc  = ones(128x128)^T @ E^T (PE; computes sum_k E and broadcasts it
                                        across all 128 partitions in one pass)
    Em^T       = E^T * mask^T          (DVE, bf16; mask is the precomputed {0,1}
                                        dropout keep-mask, transposed on host)
    O^T[d,q]  += V^T-chunk contraction (PE: lhsT = V[k-chunk] stationary,
                                        rhs = Em^T streaming, accumulate over k)
    out        = O^T * recip(rowsum)   (DVE; 1/0.9 dropout scale is folded into
                                        V on the host)
Host returns out^T transposed back to [B,H,S,D].
"""

import numpy as np
import ml_dtypes

B, H, S, D = 2, 16, 2048, 128
NCORES = 8
PAIRS = (B * H) // NCORES  # 4 (b,h) pairs per core
QB = 1024                  # q-block (PSUM: fp32 [128, QB] = 2 banks)
NQB = S // QB
NKT = S // 128             # 16 k-tiles of 128
PKEEP = 0.9

_cache = {}


def _get_maskT():
    """Dropout keep-mask, exactly as the reference computes it (jax threefry,
    key 42, CPU backend), transposed to [pair, k, q] and cast to bf16 {0,1}."""
    if "maskT" in _cache:
        return _cache["maskT"]
    import jax

    with jax.default_device(jax.devices("cpu")[0]):
        keep = jax.random.bernoulli(jax.random.key(42), PKEEP, (B, H, S, S))
    keep = np.asarray(keep).reshape(B * H, S, S)
    mT = np.ascontiguousarray(keep.transpose(0, 2, 1)).astype(ml_dtypes.bfloat16)
    _cache["maskT"] = mT
    return mT


def _build_nc():
    if "nc" in _cache:
        return _cache["nc"]
    from contextlib import ExitStack

    import concourse.bass as bass  # noqa: F401
    import concourse.mybir as mybir
    import concourse.tile as tile
    from concourse import bacc

    dt = mybir.dt
    nc = bacc.Bacc("TRN2", target_bir_lowering=False, debug=False, num_devices=NCORES)

    qT = nc.dram_tensor("qT", [PAIRS, D, S], dt.float16, kind="ExternalInput").ap()
    kT = nc.dram_tensor("kT", [PAIRS, D, S], dt.float16, kind="ExternalInput").ap()
    v = nc.dram_tensor("v", [PAIRS, 128, NKT * D], dt.bfloat16, kind="ExternalInput").ap()
    mT = nc.dram_tensor("mT", [PAIRS, S, S], dt.bfloat16, kind="ExternalInput").ap()
    oT = nc.dram_tensor("oT", [PAIRS, D, S], dt.float32, kind="ExternalOutput").ap()

    EXP = mybir.ActivationFunctionType.Exp

    LN = mybir.ActivationFunctionType.Ln

    with tile.TileContext(nc) as tc, ExitStack() as ctx:
        const_pool = ctx.enter_context(tc.tile_pool(name="const", bufs=1))
        ones = const_pool.tile([128, 128], dt.bfloat16)
        nc.vector.memset(ones[:], 1.0)

        qt_pool = ctx.enter_context(tc.tile_pool(name="qt", bufs=2))
        kt_pool = ctx.enter_context(tc.tile_pool(name="kt", bufs=2))
        v_pool = ctx.enter_context(tc.tile_pool(name="vt", bufs=2))
        m_pool = ctx.enter_context(tc.tile_pool(name="m", bufs=18))
        e_pool = ctx.enter_context(tc.tile_pool(name="e", bufs=4))
        ep_pool = ctx.enter_context(tc.tile_pool(name="ep", bufs=4))
        em_pool = ctx.enter_context(tc.tile_pool(name="em", bufs=5))
        r_pool = ctx.enter_context(tc.tile_pool(name="r", bufs=2))
        rb_pool = ctx.enter_context(tc.tile_pool(name="rb", bufs=2))
        osb_pool = ctx.enter_context(tc.tile_pool(name="osb", bufs=2))
        o_pool = ctx.enter_context(tc.tile_pool(name="o", bufs=2))
        ps_pool = ctx.enter_context(tc.tile_pool(name="ps", bufs=2, space="PSUM"))
        pr_pool = ctx.enter_context(tc.tile_pool(name="pr", bufs=1, space="PSUM"))
        po_pool = ctx.enter_context(tc.tile_pool(name="po", bufs=1, space="PSUM"))

        for p in range(PAIRS):
            kt_t = kt_pool.tile([D, S], dt.float16)
            qt_t = qt_pool.tile([D, S], dt.float16)
            for c in range(4):
                nc.sync.dma_start(
                    kt_t[:, c * 512 : (c + 1) * 512], kT[p][:, c * 512 : (c + 1) * 512]
                )
                nc.sync.dma_start(
                    qt_t[:, c * 512 : (c + 1) * 512], qT[p][:, c * 512 : (c + 1) * 512]
                )
            # v is pre-laid-out on the host as [pair, 128, NKT*D] so this is
            # one DMA with 4KB-contiguous rows
            v_t = v_pool.tile([128, NKT * D], dt.bfloat16)
            nc.sync.dma_start(v_t[:], v[p])
            # full-width mask tiles, loaded once per (p, kt), used by both
            # q-blocks (halves the DMA descriptor count)
            m_tiles = {}

            for qb in range(NQB):
                q0 = qb * QB
                r_ps = pr_pool.tile([128, QB], dt.float32)
                o_ps = po_pool.tile([128, QB], dt.float32)

                # Software-pipelined: emit mm1(kt) before mm2(kt-1)/rowsum so
                # the PE always has independent work while ACT/DVE produce
                # e/em for the previous k-tile. The 16 E tiles are pair-added
                # on DVE (bf16) into 8 partials so the PE rowsum stream is
                # halved.
                e_prev = None
                em_tiles = {}
                ep_tiles = {}
                for kt in range(NKT + 2):
                    if kt < NKT:
                        k0 = kt * 128
                        s_ps = ps_pool.tile([128, QB], dt.float32)
                        lhs_k = kt_t[:, k0 : k0 + 128]
                        for n in range(QB // 512):
                            nc.tensor.matmul(
                                s_ps[:, n * 512 : (n + 1) * 512],
                                lhs_k,
                                qt_t[:, q0 + n * 512 : q0 + (n + 1) * 512],
                                start=True,
                                stop=True,
                            )

                    if kt >= 2:
                        em = em_tiles.pop(kt - 2)
                        lhs_v = v_t[:, (kt - 2) * D : (kt - 1) * D]
                        for n in range(QB // 512):
                            nc.tensor.matmul(
                                o_ps[:, n * 512 : (n + 1) * 512],
                                lhs_v,
                                em[:, n * 512 : (n + 1) * 512],
                                start=(kt == 2),
                                stop=(kt == NKT + 1),
                                skip_group_check=True,
                            )
                    if kt >= 3 and (kt - 3) % 2 == 0:
                        j = (kt - 3) // 2
                        ep = ep_tiles.pop(j)
                        for n in range(QB // 512):
                            nc.tensor.matmul(
                                r_ps[:, n * 512 : (n + 1) * 512],
                                ones[:],
                                ep[:, n * 512 : (n + 1) * 512],
                                start=(j == 0),
                                stop=(j == NKT // 2 - 1),
                                skip_group_check=True,
                            )

                    if kt < NKT:
                        e_t = e_pool.tile([128, QB], dt.bfloat16)
                        nc.scalar.activation(e_t[:], s_ps[:], EXP)

                        if qb == 0:
                            m_t = m_pool.tile([128, S], dt.bfloat16)
                            nc.sync.dma_start(m_t[:], mT[p][k0 : k0 + 128, :])
                            m_tiles[kt] = m_t
                        else:
                            m_t = m_tiles[kt]
                        em_t = em_pool.tile([128, QB], dt.bfloat16)
                        nc.vector.tensor_mul(
                            em_t[:], e_t[:], m_t[:, q0 : q0 + QB]
                        )
                        em_tiles[kt] = em_t
                        if kt % 2 == 1:
                            j = kt // 2
                            ep_t = ep_pool.tile([128, QB], dt.bfloat16)
                            # a couple of the pair-adds go to the idle GpSimd
                            if j in (2, 5):
                                nc.gpsimd.tensor_add(ep_t[:], e_prev[:], e_t[:])
                            else:
                                nc.vector.tensor_add(ep_t[:], e_prev[:], e_t[:])
                            ep_tiles[j] = ep_t
                        e_prev = e_t

                # rowsum came out of the ones-matmul broadcast across all 128
                # partitions; reciprocal via the fast custom-DVE op (~18 bits,
                # plenty). O^T evacuates PSUM via ACT; normalize on idle GpSimd.
                rb_t = rb_pool.tile([128, QB], dt.float32)
                nc.vector.reciprocal_approx_fast(rb_t[:], r_ps[:])
                o_sb = osb_pool.tile([128, QB], dt.float32)
                nc.scalar.copy(o_sb[:], o_ps[:])
                o_t = o_pool.tile([128, QB], dt.float32)
                nc.gpsimd.tensor_mul(o_t[:], o_sb[:], rb_t[:])
                nc.sync.dma_start(oT[p][:, q0 : q0 + QB], o_t[:])

    nc.compile()
    _cache["nc"] = nc
    return nc


# Set by test harnesses to capture profile info: kernel() stores the
# BassKernelResults of the last run here when TRACE is True.
TRACE = False
LAST_RESULT = None


def kernel(**inputs):
    global LAST_RESULT
    from concourse.bass_utils import run_bass_kernel_spmd

    q = np.asarray(inputs["query"], dtype=np.float32).reshape(B * H, S, D)
    k = np.asarray(inputs["key"], dtype=np.float32).reshape(B * H, S, D)
    vv = np.asarray(inputs["value"], dtype=np.float32).reshape(B * H, S, D)

    qTh = q.transpose(0, 2, 1).astype(np.float16)  # [32, D, S]
    kTh = k.transpose(0, 2, 1).astype(np.float16)
    # [32, S, D] -> [32, 128, NKT*D]: vph[p, part, kt*D + d] = v[p, kt*128+part, d]
    # (matches the SBUF layout of the mm2 stationary tiles; dropout scale folded)
    vph = (
        (vv / PKEEP)
        .reshape(B * H, S // 128, 128, D)
        .transpose(0, 2, 1, 3)
        .reshape(B * H, 128, (S // 128) * D)
        .astype(ml_dtypes.bfloat16)
    )
    mTh = _get_maskT()                             # [32, S, S] bf16 {0,1}

    nc = _build_nc()

    in_maps = []
    for c in range(NCORES):
        sl = slice(c * PAIRS, (c + 1) * PAIRS)
        in_maps.append(
            {
                "qT": np.ascontiguousarray(qTh[sl]),
                "kT": np.ascontiguousarray(kTh[sl]),
                "v": np.ascontiguousarray(vph[sl]),
                "mT": np.ascontiguousarray(mTh[sl]),
            }
        )

    res = run_bass_kernel_spmd(nc, in_maps, core_ids=list(range(NCORES)), trace=TRACE)
    LAST_RESULT = res

    oT = np.concatenate([r["oT"] for r in res.results], axis=0)  # [32, D, S] fp32
    out = oT.transpose(0, 2, 1).reshape(B, H, S, D)
    return np.ascontiguousarray(out).astype(np.float32)


# revision 22
# speedup vs baseline: 1.1158x; 1.1158x over previous
"""Trainium2 Bass kernel for batched multi-head attention with deterministic dropout.

Reference computation (B=2, H=16, S=2048, D=128, fp32):
    qk   = einsum("bhqd,bhkd->bhqk", q, k)          # scores
    attn = softmax(qk, axis=-1)
    keep = bernoulli(jax.random.key(42), 0.9, attn.shape)
    attn = where(keep, attn / 0.9, 0)
    out  = einsum("bhqk,bhkd->bhqd", attn, v)

Sharding: the 32 (b,h) pairs are split across 8 NeuronCores, 4 pairs each.
Device-side pipeline per (pair, q-block) — everything in the S^T orientation so
no transposes of the 2048x2048 attention matrix are ever needed:
    S^T[k,q]   = K·Q^T                 (PE, fp16 in, fp32 PSUM)
    E^T        = exp(S^T)              (ACT, bf16 out; scores max ~62 so no
                                        max-subtraction is needed in fp32/bf16 range)
    rowsum_bc  = ones(128x128)^T @ E^T (PE; computes sum_k E and broadcasts it
                                        across all 128 partitions in one pass)
    Em^T       = E^T * mask^T          (DVE, bf16; mask is the precomputed {0,1}
                                        dropout keep-mask, transposed on host)
    O^T[d,q]  += V^T-chunk contraction (PE: lhsT = V[k-chunk] stationary,
                                        rhs = Em^T streaming, accumulate over k)
    out        = O^T * recip(rowsum)   (DVE; 1/0.9 dropout scale is folded into
                                        V on the host)
Host returns out^T transposed back to [B,H,S,D].
"""

import numpy as np
import ml_dtypes

B, H, S, D = 2, 16, 2048, 128
NCORES = 8
PAIRS = (B * H) // NCORES  # 4 (b,h) pairs per core
QB = 1024                  # q-block (PSUM: fp32 [128, QB] = 2 banks)
NQB = S // QB
NKT = S // 128             # 16 k-tiles of 128
PKEEP = 0.9

_cache = {}


def _get_maskT():
    """Dropout keep-mask, exactly as the reference computes it (jax threefry,
    key 42, CPU backend), transposed to [pair, k, q] and cast to bf16 {0,1}."""
    if "maskT" in _cache:
        return _cache["maskT"]
    import jax

    with jax.default_device(jax.devices("cpu")[0]):
        keep = jax.random.bernoulli(jax.random.key(42), PKEEP, (B, H, S, S))
    keep = np.asarray(keep).reshape(B * H, S, S)
    mT = np.ascontiguousarray(keep.transpose(0, 2, 1)).astype(ml_dtypes.bfloat16)
    _cache["maskT"] = mT
    return mT


def _build_nc():
    if "nc" in _cache:
        return _cache["nc"]
    from contextlib import ExitStack

    import concourse.bass as bass  # noqa: F401
    import concourse.mybir as mybir
    import concourse.tile as tile
    from concourse import bacc

    dt = mybir.dt
    nc = bacc.Bacc("TRN2", target_bir_lowering=False, debug=False, num_devices=NCORES)

    qT = nc.dram_tensor("qT", [PAIRS, D, S], dt.float16, kind="ExternalInput").ap()
    kT = nc.dram_tensor("kT", [PAIRS, D, S], dt.float16, kind="ExternalInput").ap()
    v = nc.dram_tensor("v", [PAIRS, 128, NKT * D], dt.bfloat16, kind="ExternalInput").ap()
    mT = nc.dram_tensor("mT", [PAIRS, S, S], dt.bfloat16, kind="ExternalInput").ap()
    oT = nc.dram_tensor("oT", [PAIRS, D, S], dt.float32, kind="ExternalOutput").ap()

    EXP = mybir.ActivationFunctionType.Exp

    LN = mybir.ActivationFunctionType.Ln

    with tile.TileContext(nc) as tc, ExitStack() as ctx:
        const_pool = ctx.enter_context(tc.tile_pool(name="const", bufs=1))
        ones = const_pool.tile([128, 128], dt.bfloat16)
        nc.vector.memset(ones[:], 1.0)

        qt_pool = ctx.enter_context(tc.tile_pool(name="qt", bufs=2))
        kt_pool = ctx.enter_context(tc.tile_pool(name="kt", bufs=2))
        v_pool = ctx.enter_context(tc.tile_pool(name="vt", bufs=2))
        m_pool = ctx.enter_context(tc.tile_pool(name="m", bufs=18))
        e_pool = ctx.enter_context(tc.tile_pool(name="e", bufs=4))
        ep_pool = ctx.enter_context(tc.tile_pool(name="ep", bufs=4))
        em_pool = ctx.enter_context(tc.tile_pool(name="em", bufs=5))
        r_pool = ctx.enter_context(tc.tile_pool(name="r", bufs=2))
        rb_pool = ctx.enter_context(tc.tile_pool(name="rb", bufs=2))
        osb_pool = ctx.enter_context(tc.tile_pool(name="osb", bufs=2))
        o_pool = ctx.enter_context(tc.tile_pool(name="o", bufs=2))
        ps_pool = ctx.enter_context(tc.tile_pool(name="ps", bufs=2, space="PSUM"))
        pr_pool = ctx.enter_context(tc.tile_pool(name="pr", bufs=1, space="PSUM"))
        po_pool = ctx.enter_context(tc.tile_pool(name="po", bufs=1, space="PSUM"))

        for p in range(PAIRS):
            kt_t = kt_pool.tile([D, S], dt.float16)
            qt_t = qt_pool.tile([D, S], dt.float16)
            for c in range(4):
                nc.sync.dma_start(
                    kt_t[:, c * 512 : (c + 1) * 512], kT[p][:, c * 512 : (c + 1) * 512]
                )
                nc.sync.dma_start(
                    qt_t[:, c * 512 : (c + 1) * 512], qT[p][:, c * 512 : (c + 1) * 512]
                )
            # v is pre-laid-out on the host as [pair, 128, NKT*D] so this is
            # one DMA with 4KB-contiguous rows
            v_t = v_pool.tile([128, NKT * D], dt.bfloat16)
            nc.sync.dma_start(v_t[:], v[p])
            # full-width mask tiles, loaded once per (p, kt), used by both
            # q-blocks (halves the DMA descriptor count)
            m_tiles = {}

            for qb in range(NQB):
                q0 = qb * QB
                r_ps = pr_pool.tile([128, QB], dt.float32)
                o_ps = po_pool.tile([128, QB], dt.float32)

                # Software-pipelined: emit mm1(kt) before mm2(kt-1)/rowsum so
                # the PE always has independent work while ACT/DVE produce
                # e/em for the previous k-tile. The 16 E tiles are pair-added
                # on DVE (bf16) into 8 partials so the PE rowsum stream is
                # halved.
                e_prev = None
                em_tiles = {}
                ep_tiles = {}
                for kt in range(NKT + 2):
                    if kt < NKT:
                        k0 = kt * 128
                        s_ps = ps_pool.tile([128, QB], dt.float32)
                        lhs_k = kt_t[:, k0 : k0 + 128]
                        for n in range(QB // 512):
                            nc.tensor.matmul(
                                s_ps[:, n * 512 : (n + 1) * 512],
                                lhs_k,
                                qt_t[:, q0 + n * 512 : q0 + (n + 1) * 512],
                                start=True,
                                stop=True,
                            )

                    if kt >= 2:
                        em = em_tiles.pop(kt - 2)
                        lhs_v = v_t[:, (kt - 2) * D : (kt - 1) * D]
                        for n in range(QB // 512):
                            nc.tensor.matmul(
                                o_ps[:, n * 512 : (n + 1) * 512],
                                lhs_v,
                                em[:, n * 512 : (n + 1) * 512],
                                start=(kt == 2),
                                stop=(kt == NKT + 1),
                                skip_group_check=True,
                            )
                    if kt >= 3 and (kt - 3) % 2 == 0:
                        j = (kt - 3) // 2
                        ep = ep_tiles.pop(j)
                        for n in range(QB // 512):
                            nc.tensor.matmul(
                                r_ps[:, n * 512 : (n + 1) * 512],
                                ones[:],
                                ep[:, n * 512 : (n + 1) * 512],
                                start=(j == 0),
                                stop=(j == NKT // 2 - 1),
                                skip_group_check=True,
                            )

                    if kt < NKT:
                        e_t = e_pool.tile([128, QB], dt.bfloat16)
                        nc.scalar.activation(e_t[:], s_ps[:], EXP)

                        if qb == 0:
                            m_t = m_pool.tile([128, S], dt.bfloat16)
                            nc.sync.dma_start(m_t[:], mT[p][k0 : k0 + 128, :])
                            m_tiles[kt] = m_t
                        else:
                            m_t = m_tiles[kt]
                        em_t = em_pool.tile([128, QB], dt.bfloat16)
                        nc.vector.tensor_mul(
                            em_t[:], e_t[:], m_t[:, q0 : q0 + QB]
                        )
                        em_tiles[kt] = em_t
                        if kt % 2 == 1:
                            j = kt // 2
                            ep_t = ep_pool.tile([128, QB], dt.bfloat16)
                            nc.vector.tensor_add(ep_t[:], e_prev[:], e_t[:])
                            ep_tiles[j] = ep_t
                        e_prev = e_t

                # rowsum came out of the ones-matmul broadcast across all 128
                # partitions; reciprocal via the fast custom-DVE op (~18 bits,
                # plenty). O^T evacuates PSUM via ACT; normalize on idle GpSimd.
                rb_t = rb_pool.tile([128, QB], dt.float32)
                nc.vector.reciprocal_approx_fast(rb_t[:], r_ps[:])
                o_sb = osb_pool.tile([128, QB], dt.float32)
                nc.scalar.copy(o_sb[:], o_ps[:])
                o_t = o_pool.tile([128, QB], dt.float32)
                nc.gpsimd.tensor_mul(o_t[:], o_sb[:], rb_t[:])
                nc.sync.dma_start(oT[p][:, q0 : q0 + QB], o_t[:])

    nc.compile()
    _cache["nc"] = nc
    return nc


# Set by test harnesses to capture profile info: kernel() stores the
# BassKernelResults of the last run here when TRACE is True.
TRACE = False
LAST_RESULT = None


def kernel(**inputs):
    global LAST_RESULT
    from concourse.bass_utils import run_bass_kernel_spmd

    q = np.asarray(inputs["query"], dtype=np.float32).reshape(B * H, S, D)
    k = np.asarray(inputs["key"], dtype=np.float32).reshape(B * H, S, D)
    vv = np.asarray(inputs["value"], dtype=np.float32).reshape(B * H, S, D)

    qTh = q.transpose(0, 2, 1).astype(np.float16)  # [32, D, S]
    kTh = k.transpose(0, 2, 1).astype(np.float16)
    # [32, S, D] -> [32, 128, NKT*D]: vph[p, part, kt*D + d] = v[p, kt*128+part, d]
    # (matches the SBUF layout of the mm2 stationary tiles; dropout scale folded)
    vph = (
        (vv / PKEEP)
        .reshape(B * H, S // 128, 128, D)
        .transpose(0, 2, 1, 3)
        .reshape(B * H, 128, (S // 128) * D)
        .astype(ml_dtypes.bfloat16)
    )
    mTh = _get_maskT()                             # [32, S, S] bf16 {0,1}

    nc = _build_nc()

    in_maps = []
    for c in range(NCORES):
        sl = slice(c * PAIRS, (c + 1) * PAIRS)
        in_maps.append(
            {
                "qT": np.ascontiguousarray(qTh[sl]),
                "kT": np.ascontiguousarray(kTh[sl]),
                "v": np.ascontiguousarray(vph[sl]),
                "mT": np.ascontiguousarray(mTh[sl]),
            }
        )

    res = run_bass_kernel_spmd(nc, in_maps, core_ids=list(range(NCORES)), trace=TRACE)
    LAST_RESULT = res

    oT = np.concatenate([r["oT"] for r in res.results], axis=0)  # [32, D, S] fp32
    out = oT.transpose(0, 2, 1).reshape(B, H, S, D)
    return np.ascontiguousarray(out).astype(np.float32)


# revision 25
# speedup vs baseline: 1.1425x; 1.0239x over previous
"""Trainium2 Bass kernel for batched multi-head attention with deterministic dropout.

Reference computation (B=2, H=16, S=2048, D=128, fp32):
    qk   = einsum("bhqd,bhkd->bhqk", q, k)          # scores
    attn = softmax(qk, axis=-1)
    keep = bernoulli(jax.random.key(42), 0.9, attn.shape)
    attn = where(keep, attn / 0.9, 0)
    out  = einsum("bhqk,bhkd->bhqd", attn, v)

Sharding: the 32 (b,h) pairs are split across 8 NeuronCores, 4 pairs each.
Device-side pipeline per (pair, q-block) — everything in the S^T orientation so
no transposes of the 2048x2048 attention matrix are ever needed:
    S^T[k,q]   = K·Q^T                 (PE, fp16 in, fp32 PSUM)
    E^T        = exp(S^T)              (ACT, bf16 out; scores max ~62 so no
                                        max-subtraction is needed in fp32/bf16 range)
    rowsum_bc  = ones(128x128)^T @ E^T (PE; computes sum_k E and broadcasts it
                                        across all 128 partitions in one pass)
    Em^T       = E^T * mask^T          (DVE, bf16; mask is the precomputed {0,1}
                                        dropout keep-mask, transposed on host)
    O^T[d,q]  += V^T-chunk contraction (PE: lhsT = V[k-chunk] stationary,
                                        rhs = Em^T streaming, accumulate over k)
    out        = O^T * recip(rowsum)   (DVE; 1/0.9 dropout scale is folded into
                                        V on the host)
Host returns out^T transposed back to [B,H,S,D].
"""

import numpy as np
import ml_dtypes

B, H, S, D = 2, 16, 2048, 128
NCORES = 8
PAIRS = (B * H) // NCORES  # 4 (b,h) pairs per core
QB = 1024                  # q-block (PSUM: fp32 [128, QB] = 2 banks)
NQB = S // QB
NKT = S // 128             # 16 k-tiles of 128
PKEEP = 0.9

_cache = {}


def _get_maskT():
    """Dropout keep-mask, exactly as the reference computes it (jax threefry,
    key 42, CPU backend), transposed to [pair, k, q] and cast to bf16 {0,1}."""
    if "maskT" in _cache:
        return _cache["maskT"]
    import jax

    with jax.default_device(jax.devices("cpu")[0]):
        keep = jax.random.bernoulli(jax.random.key(42), PKEEP, (B, H, S, S))
    keep = np.asarray(keep).reshape(B * H, S, S)
    mT = np.ascontiguousarray(keep.transpose(0, 2, 1)).astype(ml_dtypes.bfloat16)
    _cache["maskT"] = mT
    return mT


def _build_nc():
    if "nc" in _cache:
        return _cache["nc"]
    from contextlib import ExitStack

    import concourse.bass as bass  # noqa: F401
    import concourse.mybir as mybir
    import concourse.tile as tile
    from concourse import bacc

    dt = mybir.dt
    nc = bacc.Bacc("TRN2", target_bir_lowering=False, debug=False, num_devices=NCORES)

    qT = nc.dram_tensor("qT", [PAIRS, D, S], dt.float16, kind="ExternalInput").ap()
    kT = nc.dram_tensor("kT", [PAIRS, D, S], dt.float16, kind="ExternalInput").ap()
    v = nc.dram_tensor("v", [PAIRS, 128, NKT * D], dt.bfloat16, kind="ExternalInput").ap()
    mT = nc.dram_tensor("mT", [PAIRS, S, S], dt.bfloat16, kind="ExternalInput").ap()
    oT = nc.dram_tensor("oT", [PAIRS, D, S], dt.float32, kind="ExternalOutput").ap()

    EXP = mybir.ActivationFunctionType.Exp

    LN = mybir.ActivationFunctionType.Ln

    with tile.TileContext(nc) as tc, ExitStack() as ctx:
        const_pool = ctx.enter_context(tc.tile_pool(name="const", bufs=1))
        ones = const_pool.tile([128, 128], dt.bfloat16)
        nc.vector.memset(ones[:], 1.0)

        qt_pool = ctx.enter_context(tc.tile_pool(name="qt", bufs=2))
        kt_pool = ctx.enter_context(tc.tile_pool(name="kt", bufs=2))
        v_pool = ctx.enter_context(tc.tile_pool(name="vt", bufs=2))
        m_pool = ctx.enter_context(tc.tile_pool(name="m", bufs=18))
        e_pool = ctx.enter_context(tc.tile_pool(name="e", bufs=4))
        ep_pool = ctx.enter_context(tc.tile_pool(name="ep", bufs=5))
        em_pool = ctx.enter_context(tc.tile_pool(name="em", bufs=7))
        r_pool = ctx.enter_context(tc.tile_pool(name="r", bufs=2))
        rb_pool = ctx.enter_context(tc.tile_pool(name="rb", bufs=2))
        osb_pool = ctx.enter_context(tc.tile_pool(name="osb", bufs=2))
        o_pool = ctx.enter_context(tc.tile_pool(name="o", bufs=2))
        ps_pool = ctx.enter_context(tc.tile_pool(name="ps", bufs=2, space="PSUM"))
        pr_pool = ctx.enter_context(tc.tile_pool(name="pr", bufs=1, space="PSUM"))
        po_pool = ctx.enter_context(tc.tile_pool(name="po", bufs=1, space="PSUM"))

        for p in range(PAIRS):
            kt_t = kt_pool.tile([D, S], dt.float16)
            qt_t = qt_pool.tile([D, S], dt.float16)
            for c in range(4):
                nc.sync.dma_start(
                    kt_t[:, c * 512 : (c + 1) * 512], kT[p][:, c * 512 : (c + 1) * 512]
                )
                nc.sync.dma_start(
                    qt_t[:, c * 512 : (c + 1) * 512], qT[p][:, c * 512 : (c + 1) * 512]
                )
            # v is pre-laid-out on the host as [pair, 128, NKT*D] so this is
            # one DMA with 4KB-contiguous rows
            v_t = v_pool.tile([128, NKT * D], dt.bfloat16)
            nc.sync.dma_start(v_t[:], v[p])
            # full-width mask tiles, loaded once per (p, kt), used by both
            # q-blocks (halves the DMA descriptor count)
            m_tiles = {}

            for qb in range(NQB):
                q0 = qb * QB
                r_ps = pr_pool.tile([128, QB], dt.float32)
                o_ps = po_pool.tile([128, QB], dt.float32)

                # Software-pipelined: emit mm1(kt) before mm2(kt-1)/rowsum so
                # the PE always has independent work while ACT/DVE produce
                # e/em for the previous k-tile. The 16 E tiles are pair-added
                # on DVE (bf16) into 8 partials so the PE rowsum stream is
                # halved.
                e_prev = None
                em_tiles = {}
                ep_tiles = {}
                LAG = 4  # mm2 trails mm1 by LAG k-tiles (boundary runway)
                for kt in range(NKT + LAG):
                    if kt < NKT:
                        k0 = kt * 128
                        s_ps = ps_pool.tile([128, QB], dt.float32)
                        lhs_k = kt_t[:, k0 : k0 + 128]
                        for n in range(QB // 512):
                            nc.tensor.matmul(
                                s_ps[:, n * 512 : (n + 1) * 512],
                                lhs_k,
                                qt_t[:, q0 + n * 512 : q0 + (n + 1) * 512],
                                start=True,
                                stop=True,
                            )

                    if kt >= LAG:
                        em = em_tiles.pop(kt - LAG)
                        lhs_v = v_t[:, (kt - LAG) * D : (kt - LAG + 1) * D]
                        for n in range(QB // 512):
                            nc.tensor.matmul(
                                o_ps[:, n * 512 : (n + 1) * 512],
                                lhs_v,
                                em[:, n * 512 : (n + 1) * 512],
                                start=(kt == LAG),
                                stop=(kt == NKT + LAG - 1),
                                skip_group_check=True,
                            )
                    if kt >= LAG + 1 and (kt - LAG - 1) % 2 == 0:
                        j = (kt - LAG - 1) // 2
                        ep = ep_tiles.pop(j)
                        for n in range(QB // 512):
                            nc.tensor.matmul(
                                r_ps[:, n * 512 : (n + 1) * 512],
                                ones[:],
                                ep[:, n * 512 : (n + 1) * 512],
                                start=(j == 0),
                                stop=(j == NKT // 2 - 1),
                                skip_group_check=True,
                            )

                    if kt < NKT:
                        e_t = e_pool.tile([128, QB], dt.bfloat16)
                        nc.scalar.activation(e_t[:], s_ps[:], EXP)

                        if qb == 0:
                            m_t = m_pool.tile([128, S], dt.bfloat16)
                            nc.sync.dma_start(m_t[:], mT[p][k0 : k0 + 128, :])
                            m_tiles[kt] = m_t
                        else:
                            m_t = m_tiles[kt]
                        em_t = em_pool.tile([128, QB], dt.bfloat16)
                        nc.vector.tensor_mul(
                            em_t[:], e_t[:], m_t[:, q0 : q0 + QB]
                        )
                        em_tiles[kt] = em_t
                        if kt % 2 == 1:
                            j = kt // 2
                            ep_t = ep_pool.tile([128, QB], dt.bfloat16)
                            nc.vector.tensor_add(ep_t[:], e_prev[:], e_t[:])
                            ep_tiles[j] = ep_t
                        e_prev = e_t

                # rowsum came out of the ones-matmul broadcast across all 128
                # partitions; reciprocal via the fast custom-DVE op (~18 bits,
                # plenty). O^T evacuates PSUM via ACT; normalize on idle GpSimd.
                rb_t = rb_pool.tile([128, QB], dt.float32)
                nc.vector.reciprocal_approx_fast(rb_t[:], r_ps[:])
                o_sb = osb_pool.tile([128, QB], dt.float32)
                nc.scalar.copy(o_sb[:], o_ps[:])
                o_t = o_pool.tile([128, QB], dt.float32)
                nc.gpsimd.tensor_mul(o_t[:], o_sb[:], rb_t[:])
                nc.sync.dma_start(oT[p][:, q0 : q0 + QB], o_t[:])

    nc.compile()
    _cache["nc"] = nc
    return nc


# Set by test harnesses to capture profile info: kernel() stores the
# BassKernelResults of the last run here when TRACE is True.
TRACE = False
LAST_RESULT = None


def kernel(**inputs):
    global LAST_RESULT
    from concourse.bass_utils import run_bass_kernel_spmd

    q = np.asarray(inputs["query"], dtype=np.float32).reshape(B * H, S, D)
    k = np.asarray(inputs["key"], dtype=np.float32).reshape(B * H, S, D)
    vv = np.asarray(inputs["value"], dtype=np.float32).reshape(B * H, S, D)

    qTh = q.transpose(0, 2, 1).astype(np.float16)  # [32, D, S]
    kTh = k.transpose(0, 2, 1).astype(np.float16)
    # [32, S, D] -> [32, 128, NKT*D]: vph[p, part, kt*D + d] = v[p, kt*128+part, d]
    # (matches the SBUF layout of the mm2 stationary tiles; dropout scale folded)
    vph = (
        (vv / PKEEP)
        .reshape(B * H, S // 128, 128, D)
        .transpose(0, 2, 1, 3)
        .reshape(B * H, 128, (S // 128) * D)
        .astype(ml_dtypes.bfloat16)
    )
    mTh = _get_maskT()                             # [32, S, S] bf16 {0,1}

    nc = _build_nc()

    in_maps = []
    for c in range(NCORES):
        sl = slice(c * PAIRS, (c + 1) * PAIRS)
        in_maps.append(
            {
                "qT": np.ascontiguousarray(qTh[sl]),
                "kT": np.ascontiguousarray(kTh[sl]),
                "v": np.ascontiguousarray(vph[sl]),
                "mT": np.ascontiguousarray(mTh[sl]),
            }
        )

    res = run_bass_kernel_spmd(nc, in_maps, core_ids=list(range(NCORES)), trace=TRACE)
    LAST_RESULT = res

    oT = np.concatenate([r["oT"] for r in res.results], axis=0)  # [32, D, S] fp32
    out = oT.transpose(0, 2, 1).reshape(B, H, S, D)
    return np.ascontiguousarray(out).astype(np.float32)


# revision 26
# speedup vs baseline: 1.2499x; 1.0940x over previous
"""Trainium2 Bass kernel for batched multi-head attention with deterministic dropout.

Reference computation (B=2, H=16, S=2048, D=128, fp32):
    qk   = einsum("bhqd,bhkd->bhqk", q, k)          # scores
    attn = softmax(qk, axis=-1)
    keep = bernoulli(jax.random.key(42), 0.9, attn.shape)
    attn = where(keep, attn / 0.9, 0)
    out  = einsum("bhqk,bhkd->bhqd", attn, v)

Sharding: the 32 (b,h) pairs are split across 8 NeuronCores, 4 pairs each.
Device-side pipeline per (pair, q-block) — everything in the S^T orientation so
no transposes of the 2048x2048 attention matrix are ever needed:
    S^T[k,q]   = K·Q^T                 (PE, fp16 in, fp32 PSUM)
    E^T        = exp(S^T)              (ACT, bf16 out; scores max ~62 so no
                                        max-subtraction is needed in fp32/bf16 range)
    rowsum_bc  = ones(128x128)^T @ E^T (PE; computes sum_k E and broadcasts it
                                        across all 128 partitions in one pass)
    Em^T       = E^T * mask^T          (DVE, bf16; mask is the precomputed {0,1}
                                        dropout keep-mask, transposed on host)
    O^T[d,q]  += V^T-chunk contraction (PE: lhsT = V[k-chunk] stationary,
                                        rhs = Em^T streaming, accumulate over k)
    out        = O^T * recip(rowsum)   (DVE; 1/0.9 dropout scale is folded into
                                        V on the host)
Host returns out^T transposed back to [B,H,S,D].
"""

import numpy as np
import ml_dtypes

B, H, S, D = 2, 16, 2048, 128
NCORES = 8
PAIRS = (B * H) // NCORES  # 4 (b,h) pairs per core
QB = 1024                  # q-block (PSUM: fp32 [128, QB] = 2 banks)
NQB = S // QB
NKT = S // 128             # 16 k-tiles of 128
PKEEP = 0.9

_cache = {}


def _get_maskT():
    """Dropout keep-mask, exactly as the reference computes it (jax threefry,
    key 42, CPU backend), transposed to [pair, k, q] and cast to bf16 {0,1}."""
    if "maskT" in _cache:
        return _cache["maskT"]
    import jax

    with jax.default_device(jax.devices("cpu")[0]):
        keep = jax.random.bernoulli(jax.random.key(42), PKEEP, (B, H, S, S))
    keep = np.asarray(keep).reshape(B * H, S, S)
    mT = np.ascontiguousarray(keep.transpose(0, 2, 1)).astype(ml_dtypes.bfloat16)
    _cache["maskT"] = mT
    return mT


def _build_nc():
    if "nc" in _cache:
        return _cache["nc"]
    from contextlib import ExitStack

    import concourse.bass as bass  # noqa: F401
    import concourse.mybir as mybir
    import concourse.tile as tile
    from concourse import bacc

    dt = mybir.dt
    nc = bacc.Bacc("TRN2", target_bir_lowering=False, debug=False, num_devices=NCORES)

    qT = nc.dram_tensor("qT", [PAIRS, D, S], dt.float16, kind="ExternalInput").ap()
    kT = nc.dram_tensor("kT", [PAIRS, D, S], dt.float16, kind="ExternalInput").ap()
    v = nc.dram_tensor("v", [PAIRS, 128, NKT * D], dt.bfloat16, kind="ExternalInput").ap()
    mT = nc.dram_tensor("mT", [PAIRS, S, S], dt.bfloat16, kind="ExternalInput").ap()
    oT = nc.dram_tensor("oT", [PAIRS, D, S], dt.float32, kind="ExternalOutput").ap()

    EXP = mybir.ActivationFunctionType.Exp

    LN = mybir.ActivationFunctionType.Ln

    with tile.TileContext(nc) as tc, ExitStack() as ctx:
        const_pool = ctx.enter_context(tc.tile_pool(name="const", bufs=1))
        ones = const_pool.tile([128, 128], dt.bfloat16)
        nc.vector.memset(ones[:], 1.0)

        qt_pool = ctx.enter_context(tc.tile_pool(name="qt", bufs=2))
        kt_pool = ctx.enter_context(tc.tile_pool(name="kt", bufs=2))
        v_pool = ctx.enter_context(tc.tile_pool(name="vt", bufs=2))
        m_pool = ctx.enter_context(tc.tile_pool(name="m", bufs=18))
        e_pool = ctx.enter_context(tc.tile_pool(name="e", bufs=4))
        ep_pool = ctx.enter_context(tc.tile_pool(name="ep", bufs=5))
        em_pool = ctx.enter_context(tc.tile_pool(name="em", bufs=7))
        r_pool = ctx.enter_context(tc.tile_pool(name="r", bufs=2))
        rb_pool = ctx.enter_context(tc.tile_pool(name="rb", bufs=2))
        osb_pool = ctx.enter_context(tc.tile_pool(name="osb", bufs=2))
        o_pool = ctx.enter_context(tc.tile_pool(name="o", bufs=2))
        ps_pool = ctx.enter_context(tc.tile_pool(name="ps", bufs=2, space="PSUM"))
        pr_pool = ctx.enter_context(tc.tile_pool(name="pr", bufs=1, space="PSUM"))
        po_pool = ctx.enter_context(tc.tile_pool(name="po", bufs=1, space="PSUM"))

        for p in range(PAIRS):
            kt_t = kt_pool.tile([D, S], dt.float16)
            qt_t = qt_pool.tile([D, S], dt.float16)
            for c in range(4):
                nc.sync.dma_start(
                    kt_t[:, c * 512 : (c + 1) * 512], kT[p][:, c * 512 : (c + 1) * 512]
                )
                nc.sync.dma_start(
                    qt_t[:, c * 512 : (c + 1) * 512], qT[p][:, c * 512 : (c + 1) * 512]
                )
            # v is pre-laid-out on the host as [pair, 128, NKT*D] so this is
            # one DMA with 4KB-contiguous rows
            v_t = v_pool.tile([128, NKT * D], dt.bfloat16)
            nc.sync.dma_start(v_t[:], v[p])
            # full-width mask tiles, loaded once per (p, kt), used by both
            # q-blocks (halves the DMA descriptor count)
            m_tiles = {}

            for qb in range(NQB):
                q0 = qb * QB
                r_ps = pr_pool.tile([128, QB], dt.float32)
                o_ps = po_pool.tile([128, QB], dt.float32)

                # Software-pipelined: emit mm1(kt) before mm2(kt-1)/rowsum so
                # the PE always has independent work while ACT/DVE produce
                # e/em for the previous k-tile. The 16 E tiles are pair-added
                # on DVE (bf16) into 8 partials so the PE rowsum stream is
                # halved.
                e_prev = None
                em_tiles = {}
                ep_tiles = {}
                LAG = 4  # mm2 trails mm1 by LAG k-tiles (boundary runway)
                for kt in range(NKT + LAG):
                    if kt < NKT:
                        k0 = kt * 128
                        s_ps = ps_pool.tile([128, QB], dt.float32)
                        lhs_k = kt_t[:, k0 : k0 + 128]
                        for n in range(QB // 512):
                            nc.tensor.matmul(
                                s_ps[:, n * 512 : (n + 1) * 512],
                                lhs_k,
                                qt_t[:, q0 + n * 512 : q0 + (n + 1) * 512],
                                start=True,
                                stop=True,
                            )

                    if kt >= LAG:
                        em = em_tiles.pop(kt - LAG)
                        lhs_v = v_t[:, (kt - LAG) * D : (kt - LAG + 1) * D]
                        for n in range(QB // 512):
                            nc.tensor.matmul(
                                o_ps[:, n * 512 : (n + 1) * 512],
                                lhs_v,
                                em[:, n * 512 : (n + 1) * 512],
                                start=(kt == LAG),
                                stop=(kt == NKT + LAG - 1),
                                skip_group_check=True,
                            )
                    if kt >= LAG + 1 and (kt - LAG - 1) % 2 == 0:
                        j = (kt - LAG - 1) // 2
                        ep = ep_tiles.pop(j)
                        for n in range(QB // 512):
                            nc.tensor.matmul(
                                r_ps[:, n * 512 : (n + 1) * 512],
                                ones[:],
                                ep[:, n * 512 : (n + 1) * 512],
                                start=(j == 0),
                                stop=(j == NKT // 2 - 1),
                                skip_group_check=True,
                            )

                    if kt < NKT:
                        e_t = e_pool.tile([128, QB], dt.bfloat16)
                        nc.scalar.activation(e_t[:], s_ps[:], EXP)

                        if qb == 0:
                            m_t = m_pool.tile([128, S], dt.bfloat16)
                            nc.sync.dma_start(m_t[:], mT[p][k0 : k0 + 128, :])
                            m_tiles[kt] = m_t
                        else:
                            m_t = m_tiles[kt]
                        em_t = em_pool.tile([128, QB], dt.bfloat16)
                        nc.vector.tensor_mul(
                            em_t[:], e_t[:], m_t[:, q0 : q0 + QB]
                        )
                        em_tiles[kt] = em_t
                        if kt % 2 == 1:
                            j = kt // 2
                            ep_t = ep_pool.tile([128, QB], dt.bfloat16)
                            nc.vector.tensor_add(ep_t[:], e_prev[:], e_t[:])
                            ep_tiles[j] = ep_t
                        e_prev = e_t

                # rowsum came out of the ones-matmul broadcast across all 128
                # partitions; reciprocal via the fast custom-DVE op (~18 bits,
                # plenty). Evict + normalize O^T in a single DVE op — the
                # LAG-deep pipeline gives the next block enough runway that
                # freeing PSUM at +2.5us doesn't stall the PE. GpSimd is kept
                # idle on purpose: its SBUF port lock starves the DVE.
                rb_t = rb_pool.tile([128, QB], dt.float32)
                nc.vector.reciprocal_approx_fast(rb_t[:], r_ps[:])
                o_t = o_pool.tile([128, QB], dt.float32)
                nc.vector.tensor_mul(o_t[:], o_ps[:], rb_t[:])
                nc.sync.dma_start(oT[p][:, q0 : q0 + QB], o_t[:])

    nc.compile()
    _cache["nc"] = nc
    return nc


# Set by test harnesses to capture profile info: kernel() stores the
# BassKernelResults of the last run here when TRACE is True.
TRACE = False
LAST_RESULT = None


def kernel(**inputs):
    global LAST_RESULT
    from concourse.bass_utils import run_bass_kernel_spmd

    q = np.asarray(inputs["query"], dtype=np.float32).reshape(B * H, S, D)
    k = np.asarray(inputs["key"], dtype=np.float32).reshape(B * H, S, D)
    vv = np.asarray(inputs["value"], dtype=np.float32).reshape(B * H, S, D)

    qTh = q.transpose(0, 2, 1).astype(np.float16)  # [32, D, S]
    kTh = k.transpose(0, 2, 1).astype(np.float16)
    # [32, S, D] -> [32, 128, NKT*D]: vph[p, part, kt*D + d] = v[p, kt*128+part, d]
    # (matches the SBUF layout of the mm2 stationary tiles; dropout scale folded)
    vph = (
        (vv / PKEEP)
        .reshape(B * H, S // 128, 128, D)
        .transpose(0, 2, 1, 3)
        .reshape(B * H, 128, (S // 128) * D)
        .astype(ml_dtypes.bfloat16)
    )
    mTh = _get_maskT()                             # [32, S, S] bf16 {0,1}

    nc = _build_nc()

    in_maps = []
    for c in range(NCORES):
        sl = slice(c * PAIRS, (c + 1) * PAIRS)
        in_maps.append(
            {
                "qT": np.ascontiguousarray(qTh[sl]),
                "kT": np.ascontiguousarray(kTh[sl]),
                "v": np.ascontiguousarray(vph[sl]),
                "mT": np.ascontiguousarray(mTh[sl]),
            }
        )

    res = run_bass_kernel_spmd(nc, in_maps, core_ids=list(range(NCORES)), trace=TRACE)
    LAST_RESULT = res

    oT = np.concatenate([r["oT"] for r in res.results], axis=0)  # [32, D, S] fp32
    out = oT.transpose(0, 2, 1).reshape(B, H, S, D)
    return np.ascontiguousarray(out).astype(np.float32)
